# revision 1
# baseline (speedup 1.0000x reference)
"""Multi-head masked self-attention on 8 TRN2 NeuronCores.

Problem: B=4, S=2048, D=1024, H=16 heads (hd=64), fp32.
  q,k,v = x@W* + b*; causal softmax(q k^T / 8) @ v; out = ctx @ Wo + bo.

Sharding: core c -> (batch b = c//2, head-group g = c%2 of 8 heads).
Each core computes a partial output projection over its 512 hidden dims;
the host sums the two partials per batch and adds bo.

On-device layout strategy (no on-device transposes needed):
  - host passes xT = x[b].T  [D, S]
  - q^T, k^T computed directly as [512, S] (lhsT = W chunk, rhs = xT chunk)
  - v computed in natural [S, 512] layout (lhsT = xT chunk, rhs = Wv chunk),
    stored interleaved with a ones-column per head ("v_aug", [S, 8*65]) so the
    ctx matmul accumulates the softmax denominator in PSUM row 64 for free
  - scores are computed transposed: st[sk, sq] = k q^T; exp via ACT (softmax
    without max-subtraction: scores are O(+-10), far from fp32 overflow);
    causal masking by multiplying diagonal blocks with precomputed 0/1 masks
    and skipping fully-masked blocks
  - ctx^T[hd, sq] accumulated in PSUM = v_aug^T.T @ exp; normalization:
    fast approximate reciprocal of the den row on DVE, gpsimd
    partition_broadcast, multiply on DVE during evacuation
  - output projection uses ctx^T directly as lhsT (again no transpose);
    ctx^T aliases qT's storage (each qT j-tile dies as its wave completes)
  - all matmul operands are float32r (4x PE throughput vs fp32, ~1e-4 rel
    error); QKV projections, attention waves, and the output projection
    are emitted interleaved j-major so the scheduler keeps the PE dense
    and the HAM clock stays at 2.4 GHz
"""

import numpy as np

import concourse.bass as bass
import concourse.mybir as mybir
import concourse.tile as tile
from concourse import bacc
from concourse.bass import ts
from concourse.bass_utils import run_bass_kernel_spmd

F32 = mybir.dt.float32
F32R = mybir.dt.float32r
AF = mybir.ActivationFunctionType

B, S, D, H, HD = 4, 2048, 1024, 16, 64
G = 2                 # head groups (cores per batch)
DH = D // G           # hidden dims per core = 512
HPC = H // G          # heads per core = 8
NCORES = 8

NSQ = S // 512        # 4 sq tiles of 512
NSK = S // 128        # 16 sk chunks of 128
NFC = D // 128        # 8 feature chunks
NOC = DH // 128       # 4 out-dim chunks of the per-core hidden


def _mm(nc, out, lhsT, rhs, start, stop):
    nc.tensor.matmul(out, lhsT, rhs, start=start, stop=stop)


def build_program(f32r=True):
    """Build the single-core SPMD Bass program (same program on all 8 cores)."""
    nc = bacc.Bacc("TRN2", target_bir_lowering=False, debug=False)
    MMDT = F32R if f32r else F32  # dtype of every matmul operand

    xT_d = nc.dram_tensor("xT", [D, S], MMDT, kind="ExternalInput").ap()
    wq_d = nc.dram_tensor("wq", [D, DH], MMDT, kind="ExternalInput").ap()
    wk_d = nc.dram_tensor("wk", [D, DH], MMDT, kind="ExternalInput").ap()
    wv_d = nc.dram_tensor("wv", [D, DH], MMDT, kind="ExternalInput").ap()
    wo_d = nc.dram_tensor("wo", [DH, D], MMDT, kind="ExternalInput").ap()
    bqt_d = nc.dram_tensor("bqt", [128, NOC], F32, kind="ExternalInput").ap()
    bkt_d = nc.dram_tensor("bkt", [128, NOC], F32, kind="ExternalInput").ap()
    bvb_d = nc.dram_tensor("bvb", [128, HPC, HD], F32, kind="ExternalInput").ap()
    mask_d = nc.dram_tensor("masks", [128, 4, 512], MMDT, kind="ExternalInput").ap()
    po_d = nc.dram_tensor("po", [S, D], F32, kind="ExternalOutput").ap()

    with tile.TileContext(nc) as tc:
        _emit(tc, xT_d, wq_d, wk_d, wv_d, wo_d, bqt_d, bkt_d, bvb_d, mask_d,
              po_d, MMDT)
    nc.compile()
    return nc


def _emit(tc, xT_d, wq_d, wk_d, wv_d, wo_d, bqt_d, bkt_d, bvb_d, mask_d,
          po_d, MMDT):
    nc = tc.nc
    PS = bass.MemorySpace.PSUM

    with (
        tc.tile_pool(name="persist", bufs=1) as persist,
        tc.tile_pool(name="qkv", bufs=1) as qkv_pool,
        tc.tile_pool(name="exp", bufs=3) as exp_pool,
        tc.tile_pool(name="small", bufs=2) as small_pool,
        tc.tile_pool(name="ps_mm", bufs=3, space=PS) as ps_mm,
        tc.tile_pool(name="ps_ctx", bufs=2, space=PS) as ps_ctx,
    ):
        bqt = persist.tile([128, NOC], F32)
        bkt = persist.tile([128, NOC], F32)
        nc.sync.dma_start(bqt[:], bqt_d[:])
        nc.sync.dma_start(bkt[:], bkt_d[:])

        # HAM pre-warm: throwaway matmuls on zeros while input DMAs land,
        # so the PE clock is at 2.4 GHz when real work starts
        zw = persist.tile([128, 512], MMDT)
        nc.vector.memset(zw[:].bitcast(F32), 0.0)
        pwarm = ps_mm.tile([128, 2, 512], F32, name="mm")
        for i in range(88):
            _mm(nc, pwarm[:, i % 2, :], zw[:, 0:128], zw[:], True, True)

        # masks: [t0|t1] full width, [t2|t3] restricted to sq cols 256:512
        m01 = persist.tile([128, 2, 512], MMDT)
        m23 = persist.tile([128, 2, 256], MMDT)
        nc.sync.dma_start(m01[:], mask_d[:, 0:2, :])
        nc.sync.dma_start(m23[:], mask_d[:, 2:4, 256:512])
        bvb = persist.tile([128, HPC, HD], F32)

        # persistent activations.  qT doubles as ctx^T storage: wave j's
        # evacuation overwrites qT[:, :, j-tile] right after the last
        # score matmul that reads it (disjoint partition rows per head).
        qT = qkv_pool.tile([128, NOC, S], MMDT)       # q^T + bq, then ctx^T
        kT = qkv_pool.tile([128, NOC, S], MMDT)       # k^T + bk   [512, S]
        vA = qkv_pool.tile([128, NSK, HPC, HD + 1], MMDT)  # v + ones col
        nc.vector.memset(vA[:, :, :, HD:HD + 1].bitcast(F32), 1.0)

        def emit_head(h, j):
            """One (head, sq-tile) attention tile: scores, exp, ctx."""
            hp = (h % 2) * 64
            hc = h // 2
            nblk = 4 * j + 4
            ctxp = ps_ctx.tile([65, 512], F32, name="ctxp")
            qTj = qT[hp:hp + 64, hc, ts(j, 512)]
            for pk_ in range(nblk // 2):
                ik0 = 2 * pk_
                hi_diag = (ik0 == 4 * j + 2)
                w = 256 if hi_diag else 512
                cq = 256 if hi_diag else 0
                stp = ps_mm.tile([128, 2, 512], F32, name="mm")
                for b in range(2):
                    _mm(nc, stp[:, b, 0:w],
                        kT[hp:hp + 64, hc, ts(ik0 + b, 128)],
                        qTj[:, cq:512], True, True)
                e = exp_pool.tile([128, 2, 512], MMDT, name="e")
                nc.scalar.activation(e[:, :, 0:w], stp[:, :, 0:w],
                                     AF.Exp, scale=0.125)
                t = ik0 - 4 * j
                if t == 0:
                    nc.vector.tensor_mul(e[:], e[:], m01[:])
                elif t == 2:
                    nc.vector.tensor_mul(e[:, :, 0:256],
                                         e[:, :, 0:256], m23[:])
                for b in range(2):
                    _mm(nc, ctxp[:, cq:512], vA[:, ik0 + b, h, :],
                        e[:, b, 0:w],
                        pk_ == 0 and b == 0,
                        pk_ == nblk // 2 - 1 and b == 1)
            # normalize straight out of PSUM into qT's freed j-tile;
            # keeps the copy off the exp-paced ACT stream
            dst = qT[hp:hp + 64, hc, ts(j, 512)]
            denb = small_pool.tile([1, 512], F32, name="denb", bufs=1)
            nc.vector.tensor_copy(denb[:], ctxp[64:65, :])
            rec = small_pool.tile([1, 512], F32, name="rec", bufs=1)
            nc.vector.reciprocal_approx_fast(rec[:], denb[:])
            bcs = small_pool.tile([64, 512], F32, name="bcs")
            nc.gpsimd.partition_broadcast(bcs[:], rec[:])
            nc.vector.tensor_mul(dst, ctxp[0:64, :], bcs[:])
            if j < 2:
                # early waves run ACT-paced with PE duty near the HAM
                # activity threshold; a throwaway matmul pair per head
                # keeps the clock at 2.4 GHz (cold fp32r is 4x slower)
                wk_t = ps_mm.tile([128, 2, 512], F32, name="mm")
                for b in range(2):
                    _mm(nc, wk_t[:, b, :], zw[:, 0:128], zw[:], True, True)

        with (
            tc.tile_pool(name="wtsA", bufs=1) as wtsA,
            tc.tile_pool(name="xin", bufs=2) as xin,
        ):
            xts = {}
            xts[0] = xin.tile([128, NFC, 512], MMDT, name="xt")
            wq = wtsA.tile([128, NFC, DH], MMDT)
            wk = wtsA.tile([128, NFC, DH], MMDT)
            wv = wtsA.tile([128, NFC, DH], MMDT)
            # interleave x/wq chunks so the first K-chain starts ASAP
            for fc in range(NFC):
                nc.sync.dma_start(xts[0][:, fc], xT_d[ts(fc, 128), ts(0, 512)])
                nc.sync.dma_start(wq[:, fc], wq_d[ts(fc, 128), :])
            for fc in range(NFC):
                nc.sync.dma_start(wk[:, fc], wk_d[ts(fc, 128), :])
            for fc in range(NFC):
                nc.sync.dma_start(wv[:, fc], wv_d[ts(fc, 128), :])
            nc.sync.dma_start(bvb[:], bvb_d[:])

            def emit_A_group(j, g):
                """One PSUM-tile group of stage A(j): g=0..3 q/k pairs,
                g=4..5 v pairs."""
                xt = xts[j]
                if g < 4:
                    op, is_k = g // 2, g % 2
                    wt, bias, dstT = ((wk, bkt, kT) if is_k
                                      else (wq, bqt, qT))
                    pt = ps_mm.tile([128, 2, 512], F32, name="mm")
                    for half in range(2):
                        oc = 2 * op + half
                        for fc in range(NFC):
                            _mm(nc, pt[:, half, :],
                                wt[:, fc, ts(oc, 128)], xt[:, fc],
                                fc == 0, fc == NFC - 1)
                        nc.scalar.activation(dstT[:, oc, ts(j, 512)],
                                             pt[:, half, :], AF.Identity,
                                             bias=bias[:, oc:oc + 1])
                else:
                    sp = g - 4
                    pv = ps_mm.tile([128, 2, 512], F32, name="mm")
                    for half in range(2):
                        sc = 2 * sp + half
                        for fc in range(NFC):
                            _mm(nc, pv[:, half, :],
                                xt[:, fc, ts(sc, 128)], wv[:, fc],
                                fc == 0, fc == NFC - 1)
                        pv_r = pv[:, half, :].rearrange("p (h u) -> p h u",
                                                        u=HD)
                        nc.vector.tensor_add(vA[:, 4 * j + sc, :, 0:HD],
                                             pv_r, bvb[:])

            # A(j) projections then attention wave j; the scheduler
            # fills wave j's ACT-paced gaps with A(j+1) matmuls
            for j in range(NSQ):
                if j > 0:
                    xts[j] = xin.tile([128, NFC, 512], MMDT, name="xt")
                    for fc in range(NFC):
                        nc.sync.dma_start(xts[j][:, fc],
                                          xT_d[ts(fc, 128), ts(j, 512)])
                for g in range(6):
                    emit_A_group(j, g)
                for h in range(HPC):
                    emit_head(h, j)

        # wave 3 with early stage-C groups woven in, then the C tail
        with (
            tc.tile_pool(name="woC", bufs=1) as woC,
            tc.tile_pool(name="poC", bufs=3) as poC,
        ):
            wo = woC.tile([128, NOC, D], MMDT)
            for hc in range(NOC):
                nc.sync.dma_start(wo[:, hc], wo_d[ts(hc, 128), :])

            def emit_C_group(sq):
                pp = ps_mm.tile([128, 2, 512], F32, name="mm")
                for oc in range(2):
                    for hc in range(NOC):
                        _mm(nc, pp[:, oc, :], qT[:, hc, ts(sq, 128)],
                            wo[:, hc, ts(oc, 512)],
                            hc == 0, hc == NOC - 1)
                ot = poC.tile([128, 2, 512], F32, name="ot")
                nc.vector.tensor_copy(ot[:], pp[:])
                nc.sync.dma_start(
                    po_d[ts(sq, 128), :],
                    ot[:].rearrange("p a b -> p (a b)"))

            for sq in range(NSK):
                emit_C_group(sq)


def make_masks():
    p = np.arange(128)[:, None]
    c = np.arange(512)[None, :]
    m = np.empty((128, 4, 512), dtype=np.float32)
    for t in range(4):
        m[:, t, :] = (c >= p + 128 * t).astype(np.float32)
    return m


def make_in_maps(x, Wq, bq, Wk, bk, Wv, bv, Wo):
    masks = make_masks()
    in_maps = []
    for c in range(NCORES):
        b, g = c // 2, c % 2
        sl = slice(g * DH, (g + 1) * DH)
        in_maps.append({
            "xT": np.ascontiguousarray(x[b].T),
            "wq": np.ascontiguousarray(Wq[:, sl]),
            "wk": np.ascontiguousarray(Wk[:, sl]),
            "wv": np.ascontiguousarray(Wv[:, sl]),
            "wo": np.ascontiguousarray(Wo[sl, :]),
            "bqt": np.ascontiguousarray(bq[sl].reshape(NOC, 128).T),
            "bkt": np.ascontiguousarray(bk[sl].reshape(NOC, 128).T),
            "bvb": np.ascontiguousarray(
                np.broadcast_to(bv[sl].reshape(HPC, HD), (128, HPC, HD))),
            "masks": masks,
        })
    return in_maps


_CACHE = {}


def _get_program(f32r=True):
    key = ("prog", f32r)
    if key not in _CACHE:
        _CACHE[key] = build_program(f32r=f32r)
    return _CACHE[key]


def kernel(x, Wq, bq, Wk, bk, Wv, bv, Wo, bo, **run_kwargs):
    x = np.asarray(x, dtype=np.float32)
    Wq = np.asarray(Wq, dtype=np.float32)
    bq = np.asarray(bq, dtype=np.float32)
    Wk = np.asarray(Wk, dtype=np.float32)
    bk = np.asarray(bk, dtype=np.float32)
    Wv = np.asarray(Wv, dtype=np.float32)
    bv = np.asarray(bv, dtype=np.float32)
    Wo = np.asarray(Wo, dtype=np.float32)
    bo = np.asarray(bo, dtype=np.float32)

    nc = _get_program(f32r=run_kwargs.pop("f32r", True))
    in_maps = make_in_maps(x, Wq, bq, Wk, bk, Wv, bv, Wo)
    res = run_bass_kernel_spmd(nc, in_maps, list(range(NCORES)), **run_kwargs)
    out = np.empty((B, S, D), dtype=np.float32)
    for b in range(B):
        out[b] = res.results[2 * b]["po"] + res.results[2 * b + 1]["po"] + bo
    _CACHE["last_results"] = res
    return out

